# revision 9
# baseline (speedup 1.0000x reference)
"""AttentionRNNCell Trainium2 kernel (8-core SPMD).

Sharding: 8 cores = 4 batches x 2 head-groups. Core c handles batch b=c//2,
heads h0=4*(c%2) .. h0+3. All einsums are independent per (b, head) except the
final out_kernel contraction over heads, for which each core returns a partial
(contribution of its 4 heads) and the host sums the two partials per batch.

Math per (b, h):
  x = inputs[b] + sinpos                      (N=2048, D=512)
  q,k,v = x @ Wq/Wk/Wv                        (N, O=64)
  qe = elu(q)+1 ; ke = elu(k)+1
  A_mem = (qe @ mem) / (qe @ z + 1e-8)
  next_mem = mem + ke^T @ (v - A_mem) ; next_z = z + sum_n ke
  P = softmax(q k^T / sqrt(D)) causal
  A_dot = P @ v
  A = g*A_mem + (1-g)*A_dot, g = sigmoid(beta)
  out_partial = sum_h A_h @ OK_h
"""

import sys

sys.path.insert(0, "/opt/trn_rl_repo")

from contextlib import ExitStack

import ml_dtypes
import numpy as np

import concourse.bass as bass
import concourse.tile as tile
from concourse import bacc, mybir
from concourse.bass_utils import run_bass_kernel_spmd

FP32 = mybir.dt.float32
BF16 = mybir.dt.bfloat16
AF = mybir.ActivationFunctionType
OP = mybir.AluOpType

B, N, D, H, O = 4, 2048, 512, 8, 64
N_CORES = 8
HPC = 4  # heads per core
NCH = N // 128  # 16 n-chunks of 128
DCH = D // 128  # 4 d-chunks
NBL = N // 512  # 4 n-blocks of 512
SCALE = 1.0 / float(np.sqrt(np.float32(D)))
MAX_WAVELENGTH = 1000.0
DEBUG = False


def _sine_pos_T():
    """posT[i, n] bf16, (D, N): matches reference _sine_pos(n, d).T."""
    pos = np.arange(N, dtype=np.float64)
    i = np.arange(D)
    timescales = (1.0 / MAX_WAVELENGTH) ** ((2 * (i // 2)).astype(np.float64) / D)
    ang = timescales[:, None] * pos[None, :]  # (D, N)
    pt = np.where((i % 2 == 0)[:, None], np.sin(ang), np.cos(ang))
    return pt.astype(np.float32).astype(ml_dtypes.bfloat16)


def _build(nc):
    x_ap = nc.dram_tensor("x", (N, D), FP32, kind="ExternalInput").ap()
    wqk_ap = nc.dram_tensor("wqk", (4, D, 128), FP32, kind="ExternalInput").ap()
    wkv_ap = nc.dram_tensor("wkv", (D, 512), FP32, kind="ExternalInput").ap()
    memz_ap = nc.dram_tensor("memz", (2, 128, 65), FP32, kind="ExternalInput").ap()
    ok_ap = nc.dram_tensor("okern", (2, 128, O), FP32, kind="ExternalInput").ap()
    beta_ap = nc.dram_tensor("beta", (1, 1), FP32, kind="ExternalInput").ap()

    out_ap = nc.dram_tensor("out_p", (N, O), FP32, kind="ExternalOutput").ap()
    nmz_ap = nc.dram_tensor("nmz", (2, 128, 65), FP32, kind="ExternalOutput").ap()
    if DEBUG:
        dbg_a_ap = nc.dram_tensor("dbg_a", (128, NCH, HPC * O), FP32, kind="ExternalOutput").ap()
        dbg_ad_ap = nc.dram_tensor("dbg_ad", (128, NCH, 65), FP32, kind="ExternalOutput").ap()
        dbg_g_ap = nc.dram_tensor("dbg_g", (128, 1), FP32, kind="ExternalOutput").ap()

    post_ap = nc.inline_tensor(np.ascontiguousarray(_sine_pos_T()), name="post").ap()
    x_bf_ap = nc.dram_tensor("x_bf", (N, D), BF16).ap()
    a_dram_ap = nc.dram_tensor("a_dram", (N, HPC * O), BF16).ap()

    with tile.TileContext(nc) as tc, ExitStack() as ctx:
        const = ctx.enter_context(tc.tile_pool(name="const", bufs=1))
        xp = ctx.enter_context(tc.tile_pool(name="xp", bufs=4))
        qkp = ctx.enter_context(tc.tile_pool(name="qkp", bufs=1))
        kvp = ctx.enter_context(tc.tile_pool(name="kvp", bufs=1))
        stg = ctx.enter_context(tc.tile_pool(name="stg", bufs=2))
        ptp = ctx.enter_context(tc.tile_pool(name="ptp", bufs=2))
        tmp = ctx.enter_context(tc.tile_pool(name="tmp", bufs=2))
        ps_big = ctx.enter_context(tc.tile_pool(name="ps_big", bufs=1, space="PSUM"))
        ps_pv = ctx.enter_context(tc.tile_pool(name="ps_pv", bufs=2, space="PSUM"))
        ps_sm = ctx.enter_context(tc.tile_pool(name="ps_sm", bufs=2, space="PSUM"))

        # ---- gate g = sigmoid(beta), broadcast to (128, 1) f32 ----
        bsb = const.tile([1, 1], FP32)
        nc.sync.dma_start(bsb[:], beta_ap)
        enb = const.tile([1, 1], FP32)
        nc.scalar.activation(enb[:], bsb[:], AF.Exp, scale=-1.0)  # exp(-beta)
        nc.vector.tensor_scalar(enb[:], enb[:], 1.0, None, OP.add)
        nc.vector.reciprocal(enb[:], enb[:])  # g
        g_bc = const.tile([128, 1], FP32)
        nc.gpsimd.partition_broadcast(g_bc[:], enb[:])
        gm_bc = const.tile([128, 1], FP32)  # 1 - g
        nc.vector.tensor_scalar(gm_bc[:], g_bc[:], -1.0, 1.0, OP.mult, OP.add)

        # ---- x: cast to bf16 in DRAM, transpose to xT, add positional ----
        nc.gpsimd.dma_start(x_bf_ap, x_ap)  # fp32 -> bf16 cast DMA
        xT = []
        for c in range(DCH):
            xt = xp.tile([128, N], BF16, tag="xT")
            nc.sync.dma_start_transpose(xt[:], x_bf_ap[:, c * 128 : (c + 1) * 128])
            pt = tmp.tile([128, N], BF16, tag="pos")
            nc.sync.dma_start(pt[:], post_ap[c * 128 : (c + 1) * 128, :])
            nc.vector.tensor_add(xt[:], xt[:], pt[:])
            xT.append(xt)

        # ---- weights to SBUF (cast to bf16 via SWDGE) ----
        wqk_sb = const.tile([128, 4, DCH, 128], BF16)
        nc.gpsimd.dma_start(
            wqk_sb[:], wqk_ap.rearrange("g (kc kp) m -> kp g kc m", kp=128)
        )
        wkv_sb = const.tile([128, DCH, 512], BF16)
        nc.gpsimd.dma_start(wkv_sb[:], wkv_ap.rearrange("(kc kp) m -> kp kc m", kp=128))
        ok_sb = const.tile([128, 2, O], BF16)
        nc.gpsimd.dma_start(ok_sb[:], ok_ap.rearrange("c p m -> p c m"))
        memz_sb = const.tile([128, 2, 65], FP32)
        nc.sync.dma_start(memz_sb[:], memz_ap.rearrange("q p m -> p q m"))
        memz_bf = const.tile([128, 2, 65], BF16)
        nc.gpsimd.dma_start(memz_bf[:], memz_ap.rearrange("q p m -> p q m"))

        # ---- projections: qT/kT per pair (transposed), k/v natural ----
        # qT[p], kT[p]: (128, N) bf16; partitions [0:64]=head 2p, [64:128]=head 2p+1
        qT, kT, qeT = [], [], []
        for t in range(4):  # 0,1: q pairs; 2,3: k pairs
            dst = qkp.tile([128, N], BF16, tag=f"qkT{t}")
            for s4 in range(NBL):
                ps = ps_sm.tile([128, 512], FP32, tag="small")
                for kc in range(DCH):
                    nc.tensor.matmul(
                        ps[:],
                        wqk_sb[:, t, kc, :],
                        xT[kc][:, s4 * 512 : (s4 + 1) * 512],
                        start=(kc == 0),
                        stop=(kc == DCH - 1),
                    )
                nc.vector.tensor_copy(dst[:, s4 * 512 : (s4 + 1) * 512], ps[:])
            (qT if t < 2 else kT).append(dst)

        kvn = kvp.tile([128, NCH, 512], BF16)  # [k 4x64 | v 4x64] natural
        for c in range(NCH):
            ps = ps_sm.tile([128, 512], FP32, tag="small")
            for kc in range(DCH):
                nc.tensor.matmul(
                    ps[:],
                    xT[kc][:, c * 128 : (c + 1) * 128],
                    wkv_sb[:, kc, :],
                    start=(kc == 0),
                    stop=(kc == DCH - 1),
                )
            nc.vector.tensor_copy(kvn[:, c, :], ps[:])

        # ---- elu(x)+1 = exp(min(x,0)) + max(x,0) ----
        ke = kvp.tile([128, NCH, 256], BF16)  # k_elu natural, 4 heads
        nc.vector.tensor_scalar(ke[:], kvn[:, :, 0:256], 0.0, None, OP.min)
        nc.scalar.activation(ke[:], ke[:], AF.Exp)
        nc.vector.scalar_tensor_tensor(
            ke[:], kvn[:, :, 0:256], 0.0, ke[:], OP.max, OP.add
        )
        for p in range(2):
            qe = qkp.tile([128, N], BF16, tag=f"qeT{p}")
            nc.vector.tensor_scalar(qe[:], qT[p][:], 0.0, None, OP.min)
            nc.scalar.activation(qe[:], qe[:], AF.Exp)
            nc.vector.scalar_tensor_tensor(qe[:], qT[p][:], 0.0, qe[:], OP.max, OP.add)
            qeT.append(qe)

        # v_aug per head: (128, NCH, 65) bf16, col 64 = 1.0
        v_aug = []
        for h in range(HPC):
            va = kvp.tile([128, NCH, 65], BF16, tag=f"vaug{h}")
            nc.vector.tensor_copy(
                va[:, :, 0:64], kvn[:, :, 256 + h * 64 : 256 + (h + 1) * 64]
            )
            nc.vector.memset(va[:, :, 64:65], 1.0)
            v_aug.append(va)

        a_all = qkp.tile([128, NCH, HPC * O], BF16)

        for p in range(2):
            heads = (2 * p, 2 * p + 1)
            # ---- flash attention (scores transposed: m on partitions) ----
            ad_stage = [
                stg.tile([128, NCH, 65], FP32, tag=f"ad{s}", name=f"ad{p}_{s}") for s in range(2)
            ]
            for I in range(NBL):
                pvps = [
                    ps_pv.tile([128, NBL, 65], FP32, tag="pv", name=f"pv{p}_{I}_{_}") for _ in range(2)
                ]
                n_lo = I * 512
                for mg in range(2 * I + 2):
                    stage = ps_big.tile([128, 2, 2, 512], FP32, tag="stage")
                    for s in range(2):
                        sl = slice(64 * s, 64 * s + 64)
                        for mc in range(2):
                            j = 2 * mg + mc
                            nc.tensor.matmul(
                                stage[:, s, mc, :],
                                kT[p][sl, j * 128 : (j + 1) * 128],
                                qT[p][sl, n_lo : n_lo + 512],
                                start=True,
                                stop=True,
                                tile_position=(64 * s, 0),
                            )
                    pT = ptp.tile([128, 2, 2, 512], BF16, tag="pT")
                    nc.scalar.activation(pT[:], stage[:], AF.Exp, scale=SCALE)
                    for s in range(2):
                        for mc in range(2):
                            j = 2 * mg + mc
                            if j >= 4 * I:
                                # zero the above-diagonal region: keep where
                                # n_global - m_global = (512I + f) - (128j + p) >= 0
                                w = min((j - 4 * I + 1) * 128, 512)
                                blk = pT[:, s, mc, 0:w]
                                nc.gpsimd.affine_select(
                                    out=blk,
                                    in_=blk,
                                    pattern=[[1, w]],
                                    compare_op=OP.is_ge,
                                    fill=0.0,
                                    base=512 * I - 128 * j,
                                    channel_multiplier=-1,
                                )
                            for nl in range(NBL):
                                nc.tensor.matmul(
                                    pvps[s][:, nl, :],
                                    pT[:, s, mc, nl * 128 : (nl + 1) * 128],
                                    v_aug[heads[s]][:, j, :],
                                    start=(j == 0 and nl == 0),
                                    stop=(j == 4 * I + 3 and nl == NBL - 1),
                                    skip_group_check=True,
                                )
                for s in range(2):
                    nc.vector.tensor_copy(
                        ad_stage[s][:, 4 * I : 4 * I + 4, :], pvps[s][:]
                    )

            # ---- A_mem numerator/denominator: qe @ [mem|z] ----
            if DEBUG and p == 0:
                nc.sync.dma_start(dbg_ad_ap, ad_stage[0][:])
            am_stage = [
                stg.tile([128, NCH, 65], FP32, tag=f"am{s}", name=f"am{p}_{s}") for s in range(2)
            ]
            for gg in range(NBL):
                amps = [
                    ps_pv.tile([128, NBL, 65], FP32, tag="pv", name=f"amps{p}_{gg}_{_}") for _ in range(2)
                ]
                for s in range(2):
                    sl = slice(64 * s, 64 * s + 64)
                    for cc in range(4):
                        c = 4 * gg + cc
                        nc.tensor.matmul(
                            amps[s][:, cc, :],
                            qeT[p][sl, c * 128 : (c + 1) * 128],
                            memz_bf[sl, p, :],
                            start=(cc == 0),
                            stop=(cc == 3),
                            tile_position=(64 * s, 0),
                            skip_group_check=True,
                        )
                for s in range(2):
                    nc.vector.tensor_copy(
                        am_stage[s][:, 4 * gg : 4 * gg + 4, :], amps[s][:]
                    )

            # ---- epilogues per head: divisions, mix, vd ----
            vd = []
            for s in range(2):
                h = heads[s]
                rs = tmp.tile([128, NCH], FP32, tag="rs")
                nc.vector.reciprocal(rs[:], ad_stage[s][:, :, 64])
                nc.vector.tensor_scalar(rs[:], rs[:], gm_bc[:], None, OP.mult)
                rm = tmp.tile([128, NCH], FP32, tag="rm")
                nc.vector.tensor_scalar(rm[:], am_stage[s][:, :, 64], 1e-8, None, OP.add)
                nc.vector.reciprocal(rm[:], rm[:])
                amem = tmp.tile([128, NCH, O], BF16, tag="amem")
                nc.vector.tensor_tensor(
                    amem[:],
                    am_stage[s][:, :, 0:64],
                    rm[:, :, None].to_broadcast((128, NCH, O)),
                    OP.mult,
                )
                vdt = kvp.tile([128, NCH, 65], BF16, tag=f"vd{s}")
                nc.vector.scalar_tensor_tensor(
                    vdt[:, :, 0:64], amem[:], -1.0, v_aug[h][:, :, 0:64], OP.mult, OP.add
                )
                nc.vector.memset(vdt[:, :, 64:65], 1.0)
                vd.append(vdt)
                adot = tmp.tile([128, NCH, O], BF16, tag="adot")
                nc.vector.tensor_tensor(
                    adot[:],
                    ad_stage[s][:, :, 0:64],
                    rs[:, :, None].to_broadcast((128, NCH, O)),
                    OP.mult,
                )
                nc.vector.scalar_tensor_tensor(
                    a_all[:, :, h * O : (h + 1) * O],
                    amem[:],
                    g_bc[:],
                    adot[:],
                    OP.mult,
                    OP.add,
                )

            # ---- delta rule: next_memz = memz + ke^T @ [v - A_mem | 1] ----
            dps = ps_sm.tile([128, 512], FP32, tag="small", name=f"dps{p}")[:, 0:65]
            for s in range(2):
                h = heads[s]
                for c in range(NCH):
                    nc.tensor.matmul(
                        dps[64 * s : 64 * s + 64, :],
                        ke[:, c, h * 64 : (h + 1) * 64],
                        vd[s][:, c, :],
                        start=(s == 0 and c == 0),
                        stop=(s == 1 and c == NCH - 1),
                        tile_position=(0, 64 * s),
                        skip_group_check=True,
                    )
            nmz = tmp.tile([128, 65], FP32, tag="nmz")
            nc.vector.tensor_add(nmz[:], dps[:], memz_sb[:, p, :])
            nc.sync.dma_start(nmz_ap[p], nmz[:])

        if DEBUG:
            nc.gpsimd.dma_start(dbg_a_ap, a_all[:])
            nc.sync.dma_start(dbg_g_ap, g_bc[:])
        # ---- out projection: transpose A via DRAM, contract over (h,i) ----
        nc.sync.dma_start(a_dram_ap.rearrange("(c p) m -> p c m", p=128), a_all[:])
        outsb = qkp.tile([128, NCH, O], FP32, tag="qeT0", name="outsb")
        aT = []
        for t in range(2):
            at = qkp.tile([128, N], BF16, tag=f"qkT{t}", name=f"aT{t}")
            nc.sync.dma_start_transpose(at[:], a_dram_ap[:, t * 128 : (t + 1) * 128])
            aT.append(at)
        for c in range(NCH):
            ops = ps_sm.tile([128, 512], FP32, tag="small", name=f"ops{c}")[:, 0:O]
            for t in range(2):
                nc.tensor.matmul(
                    ops[:],
                    aT[t][:, c * 128 : (c + 1) * 128],
                    ok_sb[:, t, :],
                    start=(t == 0),
                    stop=(t == 1),
                )
            nc.vector.tensor_copy(outsb[:, c, :], ops[:])
        nc.sync.dma_start(out_ap.rearrange("(c p) m -> p c m", p=128), outsb[:])

    nc.compile()
    return nc


_NC_CACHE = None


def _get_nc():
    global _NC_CACHE
    if _NC_CACHE is None:
        nc = bacc.Bacc(
            "TRN2", target_bir_lowering=False, debug=False, num_devices=N_CORES
        )
        _NC_CACHE = _build(nc)
    return _NC_CACHE


def _shard(inputs, mem, z, attn_kernel, out_kernel, beta):
    """Build per-core input maps. Core c: batch c//2, heads 4*(c%2)..+3."""
    inputs = np.asarray(inputs, np.float32)
    mem = np.asarray(mem, np.float32)
    z = np.asarray(z, np.float32)
    ak = np.asarray(attn_kernel, np.float32)
    ok = np.asarray(out_kernel, np.float32)
    beta = np.asarray(beta, np.float32).reshape(1, 1)
    in_maps = []
    for c in range(N_CORES):
        b, h0 = c // 2, 4 * (c % 2)
        hs = slice(h0, h0 + HPC)
        # wqk: [q pair0, q pair1, k pair0, k pair1], each (D, 128)
        q = ak[:, hs, :, 2]  # (D, 4, O)
        k = ak[:, hs, :, 0]
        v = ak[:, hs, :, 1]
        wqk = np.stack(
            [
                q[:, 0:2].reshape(D, 128),
                q[:, 2:4].reshape(D, 128),
                k[:, 0:2].reshape(D, 128),
                k[:, 2:4].reshape(D, 128),
            ]
        )
        wkv = np.concatenate([k.reshape(D, 256), v.reshape(D, 256)], axis=1)
        memz = np.concatenate(
            [mem[b, hs], z[b, hs][:, :, None]], axis=2
        )  # (4, 64, 65)
        memz = memz.reshape(2, 128, 65)
        oks = ok[:, hs].transpose(1, 0, 2).reshape(2, 128, O)  # (h,i) major
        in_maps.append(
            dict(
                x=np.ascontiguousarray(inputs[b]),
                wqk=np.ascontiguousarray(wqk),
                wkv=np.ascontiguousarray(wkv),
                memz=np.ascontiguousarray(memz),
                okern=np.ascontiguousarray(oks),
                beta=beta,
            )
        )
    return in_maps


def kernel(inputs, mem, z, attn_kernel, out_kernel, beta, _return_res=False, _trace=False):
    nc = _get_nc()
    in_maps = _shard(inputs, mem, z, attn_kernel, out_kernel, beta)
    res = run_bass_kernel_spmd(
        nc, in_maps, core_ids=list(range(N_CORES)), trace=_trace
    )
    out = np.zeros((B, N, O), np.float32)
    next_mem = np.zeros((B, H, O, O), np.float32)
    next_z = np.zeros((B, H, O), np.float32)
    for c in range(N_CORES):
        b, h0 = c // 2, 4 * (c % 2)
        r = res.results[c]
        out[b] += r["out_p"]
        nmz = r["nmz"].reshape(HPC, 64, 65)
        next_mem[b, h0 : h0 + HPC] = nmz[:, :, :64]
        next_z[b, h0 : h0 + HPC] = nmz[:, :, 64]
    if _return_res:
        return (out, next_mem, next_z), res
    return out, next_mem, next_z
